# revision 12
# baseline (speedup 1.0000x reference)
"""Trainium2 Bass kernel for NinjaTurtleProjLinear: y = x @ (mask*W)^T + b.

Full shapes: x (8192, 2048) f32, weight (8192, 2048) f32, bias (8192,) f32,
sparse_mask (8192, 2048) f32 -> y (8192, 8192) f32.

Strategy (column-parallel over out_features, 8 cores, block-sparse):
  - The mask is banded: at [512 out x 128 in] block granularity most blocks
    are exactly zero, so (mask*W)^T block-columns that are all-zero are
    skipped — an exact optimization. The host analyzes the runtime mask
    into per-core nonzero-block schedules and builds one Bass program per
    core (each core only loads the x^T strips its blocks touch).
  - A fully-dense out-row (the gtoken row j=0) would force every k-strip
    onto one core; instead its 2048-deep GEMV is computed as width-1
    matmul groups distributed across the cores that already hold each
    k-strip, and the host sums the per-core partials into y[:, 0].
  - Operands ship as fp16 (mask is 0/1 so wm = wt*mt is exact in fp16);
    the mask multiply runs on DVE, matmuls accumulate fp32 in PSUM, DVE
    fuses the bias add into the PSUM->SBUF copy. Input DMAs ride the SP
    HWDGE ring, mask/bias/output DMAs the ACT ring.
"""
import sys

sys.path.insert(0, "/opt/trn_rl_repo")

import numpy as np

N_TOK = 8192
IN_F = 2048
OUT_F = 8192
N_CORES = 8
OUTF_SH = OUT_F // N_CORES    # 1024
P = 128
K_T = IN_F // P               # 16 k-strips of the full problem
TOK_WIN = 512                 # tokens per SBUF window
N_WIN = N_TOK // TOK_WIN      # 16
N_TT = TOK_WIN // P           # 4 token tiles per window
NB = 256                      # out_features per PSUM block (finer = better skip)
N_NB = OUTF_SH // NB          # 4
N_TOKT = N_TOK // P           # 64 token tiles total

_STATE = None


def _analyze(sparse_mask):
    """Derive per-core block schedules from the runtime mask (exact skips)."""
    dense_row = bool(sparse_mask[0].sum() > IN_F // 2)
    pats = []
    for c in range(N_CORES):
        msh = np.asarray(sparse_mask[c * OUTF_SH:(c + 1) * OUTF_SH])
        mb = msh.copy()
        if c == 0 and dense_row:
            mb[0, :] = 0.0
        klists = []
        for nb in range(N_NB):
            blk = mb[nb * NB:(nb + 1) * NB]
            klists.append([kt for kt in range(K_T)
                           if blk[:, kt * P:(kt + 1) * P].any()])
        strips = sorted(set().union(*[set(kl) for kl in klists]))
        pats.append({"klists": klists, "strips": strips, "zero_row0": c == 0 and dense_row})
    if dense_row:
        # distribute the dense-row GEMV over cores that already hold each strip
        owner = {}
        for c in range(N_CORES):
            for k in pats[c]["strips"]:
                owner.setdefault(k, c)
        for k in range(K_T):
            if k not in owner:          # strip loaded by nobody: give to core 0
                owner[k] = 0
                pats[0]["strips"] = sorted(set(pats[0]["strips"]) | {k})
        for c in range(N_CORES):
            pats[c]["y0k"] = sorted(k for k, oc in owner.items() if oc == c)
    else:
        for c in range(N_CORES):
            pats[c]["y0k"] = []
    return pats, dense_row


def _build_nc_core(pat):
    import concourse.bass as bass
    import concourse.mybir as mybir
    import concourse.tile as tile
    from concourse import bacc

    f32 = mybir.dt.float32
    f16 = mybir.dt.float16

    strips = pat["strips"]
    S = len(strips)
    slot = {k: i for i, k in enumerate(strips)}
    klists = [[slot[k] for k in kl] for kl in pat["klists"]]
    y0s = [slot[k] for k in pat["y0k"]]
    n0 = len(y0s)

    nc = bacc.Bacc(None)
    xt = nc.declare_dram_parameter("xt", [S * P, N_TOK], f16, isOutput=False)
    wt = nc.declare_dram_parameter("wt", [S * P, OUTF_SH], f16, isOutput=False)
    mt = nc.declare_dram_parameter("mt", [S * P, OUTF_SH], f16, isOutput=False)
    b = nc.declare_dram_parameter("b", [OUTF_SH], f32, isOutput=False)
    y = nc.declare_dram_parameter("y", [N_TOK, OUTF_SH], f16, isOutput=True)
    if n0:
        w0 = nc.declare_dram_parameter("w0", [n0 * P], f16, isOutput=False)
        y0p = nc.declare_dram_parameter("y0p", [1, N_TOK], f32, isOutput=True)

    xt_r = xt[:].rearrange("(s p) t -> p s t", p=P)
    wt_r = wt[:].rearrange("(s p) n -> p s n", p=P)
    mt_r = mt[:].rearrange("(s p) n -> p s n", p=P)

    with tile.TileContext(nc) as tc:
        with (
            tc.tile_pool(name="const", bufs=1) as const_pool,
            tc.tile_pool(name="stage", bufs=2) as stage_pool,
            tc.tile_pool(name="xw", bufs=4) as xpool,
            tc.tile_pool(name="out", bufs=4) as opool,
            tc.tile_pool(name="ps", bufs=6, space="PSUM") as pspool,
            tc.tile_pool(name="ps1", bufs=2, space="PSUM") as ps1pool,
        ):
            bias128 = const_pool.tile([P, OUTF_SH], f32)
            b_ap = b[:]
            b_bcast = bass.AP(tensor=b_ap.tensor, offset=b_ap.offset,
                              ap=[[0, P]] + list(b_ap.ap))
            nc.scalar.dma_start(out=bias128[:], in_=b_bcast)

            wm = const_pool.tile([P, S, OUTF_SH], f16)
            for s in range(S):
                wt_s = stage_pool.tile([P, OUTF_SH], f16, tag="wt")
                mt_s = stage_pool.tile([P, OUTF_SH], f16, tag="mt")
                nc.sync.dma_start(out=wt_s[:], in_=wt_r[:, s, :])
                nc.scalar.dma_start(out=mt_s[:], in_=mt_r[:, s, :])
                nc.vector.tensor_mul(wm[:, s, :], wt_s[:], mt_s[:])

            if n0:
                w0_sb = const_pool.tile([P, n0], f16)
                nc.sync.dma_start(
                    out=w0_sb[:], in_=w0[:].rearrange("(i p) -> p i", p=P))
                y0_sb = const_pool.tile([1, N_TOK], f32)

            def mm_group(ps_t, xwin, tt, slots, nslice):
                for j, s in enumerate(slots):
                    nc.tensor.matmul(
                        ps_t[:],
                        xwin[:, s, tt * P:(tt + 1) * P],
                        wm[:, s, nslice],
                        start=(j == 0),
                        stop=(j == len(slots) - 1),
                    )

            def drain(ps_t, nb, out_t):
                nc.vector.tensor_add(
                    out_t[:, nb * NB:(nb + 1) * NB], ps_t[:],
                    bias128[:, nb * NB:(nb + 1) * NB])

            def y0_window(xwin, w):
                # w0 column as the (1-wide) stationary operand, full 512-token
                # window as the moving operand: one N=512 matmul per strip.
                ps0 = ps1pool.tile([1, TOK_WIN], f32, tag="ps0", name=f"ps0_{w}")
                for j, s in enumerate(y0s):
                    nc.tensor.matmul(
                        ps0[:],
                        w0_sb[:, j:j + 1],
                        xwin[:, s, :],
                        start=(j == 0),
                        stop=(j == n0 - 1),
                    )
                nc.vector.tensor_copy(
                    y0_sb[:, w * TOK_WIN:(w + 1) * TOK_WIN], ps0[:])

            for w in range(N_WIN):
                xwin = xpool.tile([P, S, TOK_WIN], f16)
                nc.sync.dma_start(
                    out=xwin[:], in_=xt_r[:, :, w * TOK_WIN:(w + 1) * TOK_WIN])
                if w == 0:
                    # k-outer, one token tile (up to 4 psum groups) per chunk:
                    # matmuls start as soon as each weight strip's DMA +
                    # mask-multiply lands.
                    for tt in range(N_TT):
                        out_t = opool.tile([P, OUTF_SH], f16, tag="out_h",
                                           name=f"out_w0_{tt}")
                        sub = [nb for nb in range(N_NB) if klists[nb]]
                        pss = [pspool.tile([P, NB], f32, tag="ps",
                                           name=f"ps_w0_{tt}_{g}")
                               for g in range(len(sub))]
                        for s in range(S):
                            for g, nb in enumerate(sub):
                                kl = klists[nb]
                                if s not in kl:
                                    continue
                                j = kl.index(s)
                                nc.tensor.matmul(
                                    pss[g][:],
                                    xwin[:, s, tt * P:(tt + 1) * P],
                                    wm[:, s, nb * NB:(nb + 1) * NB],
                                    start=(j == 0),
                                    stop=(j == len(kl) - 1),
                                )
                        for g, nb in enumerate(sub):
                            drain(pss[g], nb, out_t)
                        for nb in range(N_NB):
                            if not klists[nb]:
                                nc.vector.tensor_copy(
                                    out_t[:, nb * NB:(nb + 1) * NB],
                                    bias128[:, nb * NB:(nb + 1) * NB])
                        nc.scalar.dma_start(out=y[tt * P:(tt + 1) * P, :],
                                            in_=out_t[:])
                    if n0:
                        y0_window(xwin, 0)
                    continue
                for tt in range(N_TT):
                    t0 = w * TOK_WIN + tt * P
                    out_t = opool.tile([P, OUTF_SH], f16, tag="out_h",
                                       name=f"out_{w}_{tt}")
                    for nb in range(N_NB):
                        kl = klists[nb]
                        if not kl:
                            nc.vector.tensor_copy(
                                out_t[:, nb * NB:(nb + 1) * NB],
                                bias128[:, nb * NB:(nb + 1) * NB])
                            continue
                        ps = pspool.tile([P, NB], f32, tag="ps",
                                         name=f"ps_{w}_{tt}_{nb}")
                        mm_group(ps, xwin, tt, kl, slice(nb * NB, (nb + 1) * NB))
                        drain(ps, nb, out_t)
                    nc.scalar.dma_start(out=y[t0:t0 + P, :], in_=out_t[:])
                if n0:
                    y0_window(xwin, w)
            if n0:
                nc.scalar.dma_start(out=y0p[:], in_=y0_sb[:])
    nc.compile()
    return nc


def _make_core_runner(nc):
    import jax
    import concourse.mybir as mybir
    from concourse import bass2jax

    partition_name = (nc.partition_id_tensor.name
                      if nc.partition_id_tensor else None)
    in_names, out_names, out_avals = [], [], []
    for alloc in nc.m.functions[0].allocations:
        if not isinstance(alloc, mybir.MemoryLocationSet):
            continue
        name = alloc.memorylocations[0].name
        if alloc.kind == "ExternalInput":
            if name != partition_name:
                in_names.append(name)
        elif alloc.kind == "ExternalOutput":
            out_names.append(name)
            out_avals.append(jax.core.ShapedArray(
                tuple(alloc.tensor_shape), mybir.dt.np(alloc.dtype)))
    n_params = len(in_names)
    n_outs = len(out_names)
    all_in_names = list(in_names) + list(out_names)
    if partition_name is not None:
        all_in_names = all_in_names + [partition_name]

    def _body(*args):
        operands = list(args)
        if partition_name is not None:
            operands.append(bass2jax.partition_id_tensor())
        outs = bass2jax._bass_exec_p.bind(
            *operands,
            out_avals=tuple(out_avals),
            in_names=tuple(all_in_names),
            out_names=tuple(out_names),
            lowering_input_output_aliases=(),
            sim_require_finite=True,
            sim_require_nnan=True,
            nc=nc,
        )
        return tuple(outs)

    donate = tuple(range(n_params, n_params + n_outs))
    fn = jax.jit(_body, donate_argnums=donate, keep_unused=True)
    out_shapes = [tuple(a.shape) for a in out_avals]
    out_dtypes = [a.dtype for a in out_avals]
    return fn, in_names, out_names, out_shapes, out_dtypes


def _pack_inputs(pat, c, xt16, weight, bias, sparse_mask):
    strips = pat["strips"]
    sl = slice(c * OUTF_SH, (c + 1) * OUTF_SH)
    xt_p = np.concatenate([xt16[k * P:(k + 1) * P] for k in strips], axis=0)
    wsh_t = weight[sl].T  # (IN_F, OUTF_SH)
    msh_t = np.asarray(sparse_mask[sl]).T.copy()
    if pat["zero_row0"]:
        msh_t[:, 0] = 0.0
    wt_p = np.concatenate(
        [wsh_t[k * P:(k + 1) * P] for k in strips], axis=0).astype(np.float16)
    mt_p = np.concatenate(
        [msh_t[k * P:(k + 1) * P] for k in strips], axis=0).astype(np.float16)
    ins = {"xt": np.ascontiguousarray(xt_p),
           "wt": np.ascontiguousarray(wt_p),
           "mt": np.ascontiguousarray(mt_p),
           "b": np.ascontiguousarray(bias[sl])}
    if pat["y0k"]:
        w0_full = (weight[0] * np.asarray(sparse_mask[0])).astype(np.float16)
        ins["w0"] = np.ascontiguousarray(np.concatenate(
            [w0_full[k * P:(k + 1) * P] for k in pat["y0k"]]))
    return ins


def _make_runner(sparse_mask):
    import jax

    pats, dense_row = _analyze(sparse_mask)
    runners = []
    for c in range(N_CORES):
        nc = _build_nc_core(pats[c])
        runners.append(_make_core_runner(nc))
    devices = jax.devices()[:N_CORES]
    return pats, dense_row, runners, devices


def kernel(x, weight, bias, sparse_mask):
    global _STATE
    import jax

    x = np.asarray(x, dtype=np.float32)
    weight = np.asarray(weight, dtype=np.float32)
    bias = np.asarray(bias, dtype=np.float32)
    sparse_mask = np.asarray(sparse_mask, dtype=np.float32)

    mask_key = hash(sparse_mask.tobytes())
    if _STATE is None or _STATE[0] != mask_key:
        _STATE = (mask_key, _make_runner(sparse_mask))
    _, (pats, dense_row, runners, devices) = _STATE

    xt16 = np.ascontiguousarray(x.T.astype(np.float16))
    futures = []
    for c in range(N_CORES):
        fn, in_names, out_names, out_shapes, out_dtypes = runners[c]
        ins = _pack_inputs(pats[c], c, xt16, weight, bias, sparse_mask)
        args = [jax.device_put(ins[n], devices[c]) for n in in_names]
        zeros = [jax.device_put(np.zeros(s, d), devices[c])
                 for s, d in zip(out_shapes, out_dtypes)]
        futures.append(fn(*args, *zeros))

    y_parts = []
    y0_sum = None
    for c in range(N_CORES):
        _, _, out_names, _, _ = runners[c]
        outs = futures[c]
        om = {n: outs[i] for i, n in enumerate(out_names)}
        y_parts.append(np.asarray(om["y"]).astype(np.float32))
        if "y0p" in om:
            p = np.asarray(om["y0p"]).reshape(N_TOK)
            y0_sum = p if y0_sum is None else y0_sum + p
    y_full = np.concatenate(y_parts, axis=1)
    if dense_row and y0_sum is not None:
        y_full[:, 0] = y0_sum + bias[0]
    return y_full
